# revision 44
# baseline (speedup 1.0000x reference)
"""DoubleMaskedChamferDistance Trainium2 kernel.

Full inputs: video_feat [128,512,512] f32, lang_feat [128,64,512] f32,
mask_v [128,512] f32, mask_l [128,64] f32  ->  out [128] f32.
Sharding: data-parallel over batch B=128 across 8 cores (16 per core).

Math notes (per batch):
 - pd[v,l] = |v|^2 - 2 v.l + |l|^2 ; masked = pd + (1 - mv ml) * M.
   Any constant M >= max(pd) gives identical output (pd <= ~1400; we use
   M = 32768), removing the cross-batch/cross-core global-max dependency.
 - Per batch PAIR, one PSUM accumulation in [l(2x64), v=512] layout:
   -2ab via 8 bf16 matmuls (4 d-chunks x 2 column-group halves running
   CONCURRENTLY in different PE column groups), |v|^2 via 2 ones-matmuls
   over folded squared-videoT, -M*ml[l]*mv[v] via a K=1 matmul; +(b+M)
   applied as the ACT bias at evacuation. minsl = free-dim min; minsv =
   PE-transpose + free-dim min.

Schedule notes (engines are strict in-order queues; emission ~= order):
 - Three-stage pipeline per slot j: b2_evac(j-2) FIRST (its psum dep is
   the oldest, so ACT never head-blocks), then phase_a(j) [transposes +
   evacuations + squares], phase_b(j-1) [matmuls], b2_mins(j-2).
 - Video DMA: one SWDGE cast-load (f32->bf16) per batch; 1MB loads keep
   the Q7 descriptor generator ahead of the SDMA stream (2MB pair loads
   double Q7 DRAIN stalls; measured).
 - DMA order: pair-0 batches, lang(pair 0), mask rows, constants (the
   identity iota runs on gpsimd between descriptor gens), pair 1, rest
   of lang, pairs 2-7. Engine balance per pair: PE ~ transposes+matmuls,
   DVE ~ t0-evac+sq+folds+langT+mins, ACT ~ t1-evac+sq+lang-sq+masked.

Constraint honored: every DMA carries at most ONE semaphore wait, so
DMAs only write fresh tiles; marshalling is done by compute engines.
"""

import numpy as np

import concourse.bass as bass
import concourse.mybir as mybir
import concourse.tile as tile
from concourse import bacc, masks
from concourse.bass_utils import run_bass_kernel_spmd

N_CORES = 8
B, TV, TL, D = 128, 512, 64, 512
B_LOC = B // N_CORES  # 16
NP = B_LOC // 2  # 8
M_CONST = 32768.0

F32 = mybir.dt.float32
BF16 = mybir.dt.bfloat16
AX = mybir.AxisListType


def _emit(nc, tc, ctx, video, lang, mask_v, mask_l, out):
    TT = mybir.AluOpType
    AF = mybir.ActivationFunctionType

    consts = ctx.enter_context(tc.tile_pool(name="consts", bufs=1))
    vpool = ctx.enter_context(tc.tile_pool(name="vpool", bufs=1))
    vT = ctx.enter_context(tc.tile_pool(name="vT", bufs=3))
    langp = ctx.enter_context(tc.tile_pool(name="langp", bufs=4))
    sqs = ctx.enter_context(tc.tile_pool(name="sqs", bufs=4))
    smalls = ctx.enter_context(tc.tile_pool(name="smalls", bufs=4))
    maskedp = ctx.enter_context(tc.tile_pool(name="maskedp", bufs=4))
    ps_vT = ctx.enter_context(tc.tile_pool(name="ps_vT", bufs=3, space="PSUM"))
    ps_main = ctx.enter_context(tc.tile_pool(name="ps_main", bufs=3, space="PSUM"))
    ps_small = ctx.enter_context(tc.tile_pool(name="ps_small", bufs=2, space="PSUM"))

    # ---- DMA: first pair's batches first, identity right behind ----
    vid = []

    vp0 = vpool.tile([128, 8, 512], BF16, tag="vidp0")
    nc.gpsimd.dma_start(
        out=vp0[:, 0:4], in_=video[0].rearrange("(s p) d -> p s d", p=128)
    )
    nc.gpsimd.dma_start(
        out=vp0[:, 4:8], in_=video[1].rearrange("(s p) d -> p s d", p=128)
    )
    vid.append(vp0)

    def _load_pair(j):
        t = vpool.tile([128, 8, 512], BF16, tag=f"vidp{j}")
        for tt in range(2):
            nc.gpsimd.dma_start(
                out=t[:, 4 * tt : 4 * tt + 4],
                in_=video[2 * j + tt].rearrange("(s p) d -> p s d", p=128),
            )
        vid.append(t)

    lang_bf = consts.tile([128, NP, 512], BF16)
    lang_pairs_src = lang.rearrange("(j two) l d -> (two l) j d", two=2)
    nc.gpsimd.dma_start(out=lang_bf[:, 0:1, :], in_=lang_pairs_src[:, 0:1, :])
    maskv_rows = consts.tile([1, B_LOC, 512], BF16)
    nc.gpsimd.dma_start(
        out=maskv_rows[:], in_=mask_v.rearrange("(o b) v -> o b v", o=1)
    )
    maskl_rows = consts.tile([1, B_LOC, 64], BF16)
    nc.gpsimd.dma_start(
        out=maskl_rows[:], in_=mask_l.rearrange("(o b) l -> o b l", o=1)
    )

    # constants (identity iota on gpsimd runs after the gens above)
    identf = consts.tile([128, 128], F32)
    masks.make_identity(nc, identf[:])
    identb = consts.tile([128, 128], BF16)
    masks.make_identity(nc, identb[:])
    ones128 = consts.tile([128, 1], F32)
    nc.vector.memset(ones128[:], 1.0)
    m_col = consts.tile([128, 1], F32)
    nc.vector.memset(m_col[:], M_CONST)
    ones_mat = consts.tile([128, 64], BF16)
    nc.vector.memset(ones_mat[:], 1.0)
    ones_top = consts.tile([128, 1], F32)
    nc.vector.memset(ones_top[:], 0.0)
    nc.vector.memset(ones_top[0:64], 1.0)
    ones_bot = consts.tile([128, 1], F32)
    nc.vector.memset(ones_bot[:], 0.0)
    nc.vector.memset(ones_bot[64:128], 1.0)

    _load_pair(1)
    nc.gpsimd.dma_start(out=lang_bf[:, 1:NP, :], in_=lang_pairs_src[:, 1:NP, :])

    for j in range(2, NP):
        _load_pair(j)

    # masks natural layouts (HWDGE f32)
    maskv_nat = consts.tile([B_LOC, 512], F32)
    nc.sync.dma_start(out=maskv_nat[:], in_=mask_v)
    maskl_pair_nat = consts.tile([NP, 128], F32)
    nc.sync.dma_start(
        out=maskl_pair_nat[:], in_=mask_l.rearrange("(j two) l -> j (two l)", two=2)
    )
    mvc_ps = ps_small.tile([128, 4, B_LOC], F32, tag="ps_sm")
    for s in range(4):
        nc.tensor.transpose(
            mvc_ps[:, s],
            maskv_nat[:, 128 * s : 128 * (s + 1)],
            identf[0:B_LOC, 0:B_LOC],
        )
    maskv_cols = consts.tile([128, 4, B_LOC], F32)
    nc.vector.tensor_copy(maskv_cols[:], mvc_ps[:])
    mlc_ps = ps_small.tile([128, NP], F32, tag="ps_sm")
    nc.tensor.transpose(mlc_ps[:], maskl_pair_nat[:], identf[0:NP, 0:NP])
    masklT_pair = consts.tile([128, NP], F32)
    nc.vector.tensor_copy(masklT_pair[:], mlc_ps[:])

    # mask-only normalizers
    nv_sums = consts.tile([128, B_LOC], F32)
    nc.vector.tensor_reduce(
        nv_sums[:],
        maskv_cols[:].rearrange("p s b -> p b s"),
        axis=AX.X,
        op=TT.add,
    )
    red_nv = ps_small.tile([1, B_LOC], F32, tag="ps_sm")
    nc.tensor.matmul(red_nv[:], ones128[:], nv_sums[:], start=True, stop=True)
    rv = smalls.tile([1, B_LOC], F32, tag="rv")
    nc.vector.reciprocal(rv[:], red_nv[:])
    red_nl_e = ps_small.tile([1, NP], F32, tag="ps_sm")
    nc.tensor.matmul(red_nl_e[:], ones_top[:], masklT_pair[:], start=True, stop=True)
    rl_e = smalls.tile([1, NP], F32, tag="rl_e")
    nc.vector.reciprocal(rl_e[:], red_nl_e[:])
    red_nl_o = ps_small.tile([1, NP], F32, tag="ps_sm")
    nc.tensor.matmul(red_nl_o[:], ones_bot[:], masklT_pair[:], start=True, stop=True)
    rl_o = smalls.tile([1, NP], F32, tag="rl_o")
    nc.vector.reciprocal(rl_o[:], red_nl_o[:])

    negm_rows = consts.tile([1, B_LOC, 64], BF16)
    nc.vector.tensor_scalar_mul(negm_rows[:], maskl_rows[:], -M_CONST)

    minsv_all = consts.tile([128, B_LOC, 4], BF16)
    minsl_pairs = consts.tile([128, NP], F32)
    b_pairs = consts.tile([128, NP], F32)
    bias_pairs = consts.tile([128, NP], F32)

    def phase_a(j):
        # video transposes + evacuations (t0 -> DVE, t1 -> ACT)
        vt_sb = vT.tile([128, 8, 512], BF16, tag="vt_sb")
        for t in range(2):
            for g in range(2):
                vt_ps = ps_vT.tile([128, 2, 512], BF16, tag="vt_ps")
                for kk in range(2):
                    k = 2 * g + kk
                    for s in range(4):
                        nc.tensor.transpose(
                            vt_ps[:, kk, 128 * s : 128 * (s + 1)],
                            vid[j][:, 4 * t + s, 128 * k : 128 * (k + 1)],
                            identb[:],
                        )
                if t == 0:
                    nc.vector.tensor_copy(
                        vt_sb[:, 2 * g : 2 * g + 2], vt_ps[:]
                    )
                else:
                    nc.scalar.copy(vt_sb[:, 4 + 2 * g : 4 + 2 * g + 2], vt_ps[:])

        # lang: squares+accum on ACT, bias col, langT transposes + DVE -2x
        sq_l = sqs.tile([128, 512], BF16, tag="sq_l")
        nc.scalar.activation(
            sq_l[:], lang_bf[:, j], AF.Square, accum_out=b_pairs[:, j : j + 1]
        )
        nc.scalar.activation(
            bias_pairs[:, j : j + 1],
            b_pairs[:, j : j + 1],
            AF.Identity,
            bias=m_col[:],
        )
        lg_ps = ps_small.tile([128, 4, 128], BF16, tag="ps_sm")
        for k in range(4):
            nc.tensor.transpose(
                lg_ps[:, k], lang_bf[:, j, 128 * k : 128 * (k + 1)], identb[:]
            )
        langT = langp.tile([128, 4, 128], BF16, tag="langT")
        nc.vector.tensor_scalar_mul(langT[:], lg_ps[:], -2.0)

        # squares: t0 on DVE (one FD2048 op), t1 on ACT; folds on DVE
        sq0 = sqs.tile([128, 4, 512], BF16, tag="sq0")
        nc.vector.tensor_tensor(sq0[:], vt_sb[:, 0:4], vt_sb[:, 0:4], op=TT.mult)
        sq1 = sqs.tile([128, 4, 512], BF16, tag="sq1")
        nc.scalar.activation(sq1[:], vt_sb[:, 4:8], AF.Square)
        sqh = sqs.tile([128, 2, 2, 512], BF16, tag="sqh")
        nc.vector.tensor_tensor(sqh[:, 0], sq0[:, 0:2], sq0[:, 2:4], op=TT.add)
        nc.vector.tensor_tensor(sqh[:, 1], sq1[:, 0:2], sq1[:, 2:4], op=TT.add)
        return langT, vt_sb, sqh

    def phase_b(j, actx):
        langT, vt_sb, sqh = actx
        psum_pair = ps_main.tile([128, 512], F32, tag="psum_T")
        halves = [psum_pair[0:64, :], psum_pair[64:128, :]]
        for k in range(4):
            for t in range(2):
                nc.tensor.matmul(
                    halves[t],
                    langT[:, k, 64 * t : 64 * (t + 1)],
                    vt_sb[:, 4 * t + k],
                    start=(k == 0),
                    stop=False,
                    skip_group_check=True,
                )
        for k in range(2):
            for t in range(2):
                nc.tensor.matmul(
                    halves[t],
                    ones_mat[:],
                    sqh[:, t, k],
                    start=False,
                    stop=False,
                    skip_group_check=True,
                )
        for t in range(2):
            nc.tensor.matmul(
                halves[t],
                negm_rows[:, 2 * j + t],
                maskv_rows[:, 2 * j + t],
                start=False,
                stop=True,
                skip_group_check=True,
            )
        return psum_pair

    def b2_evac(j, psum_pair):
        masked_pr = maskedp.tile([128, 512], BF16, tag="masked_pr")
        nc.scalar.activation(
            masked_pr[:],
            psum_pair[:],
            AF.Identity,
            bias=bias_pairs[:, j : j + 1],
            scale=1.0,
        )
        return masked_pr

    def b2_mins(j, masked_pr):
        nc.vector.tensor_reduce(
            minsl_pairs[:, j : j + 1], masked_pr[:], axis=AX.X, op=TT.min
        )
        o2 = ps_small.tile([128, 4, 2, 64], BF16, tag="ps_sm")
        for s in range(4):
            nc.tensor.transpose(
                o2[:, s], masked_pr[:, 128 * s : 128 * (s + 1)], identb[:]
            )
        nc.vector.tensor_reduce(
            minsv_all[:, 2 * j : 2 * j + 2, :].rearrange("p t s -> p s t"),
            o2[:],
            axis=AX.X,
            op=TT.min,
        )

    # slot order: b2_evac(j-2), a(j), b(j-1), b2_mins(j-2)
    mv_mask = consts.tile([128, B_LOC, 4], F32)
    mv_sums = consts.tile([128, B_LOC], F32)
    mlm = consts.tile([128, NP], F32)
    mvcT = maskv_cols[:].rearrange("p s b -> p b s")

    def final_partial(b0, b1):
        # masked minsv/minsl partial reductions for batches [b0:b1)
        nc.vector.tensor_tensor(
            mv_mask[:, b0:b1], minsv_all[:, b0:b1], mvcT[:, b0:b1], op=TT.mult
        )
        nc.vector.tensor_reduce(
            mv_sums[:, b0:b1], mv_mask[:, b0:b1], axis=AX.X, op=TT.add
        )
        nc.vector.tensor_tensor(
            mlm[:, b0 // 2 : b1 // 2],
            minsl_pairs[:, b0 // 2 : b1 // 2],
            masklT_pair[:, b0 // 2 : b1 // 2],
            op=TT.mult,
        )

    actx, bctx, mctx = {}, {}, {}
    for j in range(NP + 2):
        if j >= 2:
            mctx[j - 2] = b2_evac(j - 2, bctx.pop(j - 2))
        if j < NP:
            actx[j] = phase_a(j)
        if 1 <= j <= NP:
            bctx[j - 1] = phase_b(j - 1, actx.pop(j - 1))
        if j >= 2:
            b2_mins(j - 2, mctx.pop(j - 2))
        if j == NP:
            # most collector work hides under the last pairs' compute
            final_partial(0, B_LOC - 4)

    # ---- final tail: last two pairs' collectors + global reductions ----
    final_partial(B_LOC - 4, B_LOC)

    red_mv = ps_main.tile([1, B_LOC], F32, tag="psum_T")
    nc.tensor.matmul(red_mv[:], ones128[:], mv_sums[:], start=True, stop=True)
    t1 = smalls.tile([1, B_LOC], F32, tag="t1")
    nc.vector.tensor_tensor(t1[:], red_mv[:], rv[:], op=TT.mult)

    t2 = smalls.tile([1, B_LOC], F32, tag="t2")
    t2v = t2[:].rearrange("a (jj two) -> a jj two", two=2)

    red_ml_e = ps_main.tile([1, NP], F32, tag="psum_T")
    nc.tensor.matmul(red_ml_e[:], ones_top[:], mlm[:], start=True, stop=True)
    nc.vector.tensor_tensor(t2v[:, :, 0], red_ml_e[:], rl_e[:], op=TT.mult)

    red_ml_o = ps_main.tile([1, NP], F32, tag="psum_T")
    nc.tensor.matmul(red_ml_o[:], ones_bot[:], mlm[:], start=True, stop=True)
    nc.vector.tensor_tensor(t2v[:, :, 1], red_ml_o[:], rl_o[:], op=TT.mult)

    out_sb = smalls.tile([1, B_LOC], F32, tag="out_sb")
    nc.vector.tensor_tensor(out_sb[:], t1[:], t2[:], op=TT.add)
    nc.sync.dma_start(out=out[:], in_=out_sb[:])


_CACHED_NC = None


def _get_nc():
    global _CACHED_NC
    if _CACHED_NC is None:
        from contextlib import ExitStack

        nc = bacc.Bacc(
            "TRN2", target_bir_lowering=False, debug=False, num_devices=N_CORES
        )
        video = nc.dram_tensor(
            "video", [B_LOC, TV, D], F32, kind="ExternalInput"
        ).ap()
        lang = nc.dram_tensor("lang", [B_LOC, TL, D], F32, kind="ExternalInput").ap()
        mask_v = nc.dram_tensor(
            "mask_v", [B_LOC, TV], F32, kind="ExternalInput"
        ).ap()
        mask_l = nc.dram_tensor(
            "mask_l", [B_LOC, TL], F32, kind="ExternalInput"
        ).ap()
        out = nc.dram_tensor("out", [1, B_LOC], F32, kind="ExternalOutput").ap()
        with tile.TileContext(nc) as tc:
            with ExitStack() as ctx:
                _emit(nc, tc, ctx, video, lang, mask_v, mask_l, out)
        nc.compile()
        _CACHED_NC = nc
    return _CACHED_NC


def _run(video_feat, lang_feat, mask_v, mask_l, trace=False):
    nc = _get_nc()
    video_feat = np.ascontiguousarray(video_feat, dtype=np.float32)
    lang_feat = np.ascontiguousarray(lang_feat, dtype=np.float32)
    mask_v = np.ascontiguousarray(mask_v, dtype=np.float32)
    mask_l = np.ascontiguousarray(mask_l, dtype=np.float32)
    in_maps = []
    for c in range(N_CORES):
        sl = slice(c * B_LOC, (c + 1) * B_LOC)
        in_maps.append(
            {
                "video": video_feat[sl],
                "lang": lang_feat[sl],
                "mask_v": mask_v[sl],
                "mask_l": mask_l[sl],
            }
        )
    res = run_bass_kernel_spmd(nc, in_maps, list(range(N_CORES)), trace=trace)
    full = np.concatenate(
        [res.results[c]["out"].reshape(-1) for c in range(N_CORES)]
    ).astype(np.float32)
    return full, res


def kernel(video_feat, lang_feat, mask_v, mask_l):
    out, _ = _run(video_feat, lang_feat, mask_v, mask_l, trace=False)
    return out


# revision 46
# speedup vs baseline: 1.0750x; 1.0750x over previous
"""DoubleMaskedChamferDistance Trainium2 kernel.

Full inputs: video_feat [128,512,512] f32, lang_feat [128,64,512] f32,
mask_v [128,512] f32, mask_l [128,64] f32  ->  out [128] f32.
Sharding: data-parallel over batch B=128 across 8 cores (16 per core).

Math notes (per batch):
 - pd[v,l] = |v|^2 - 2 v.l + |l|^2 ; masked = pd + (1 - mv ml) * M.
   Any constant M >= max(pd) gives identical output (pd <= ~1400; we use
   M = 32768), removing the cross-batch/cross-core global-max dependency.
 - Per batch PAIR, one PSUM accumulation in [l(2x64), v=512] layout:
   -2ab via 8 bf16 matmuls (4 d-chunks x 2 column-group halves running
   CONCURRENTLY in different PE column groups), |v|^2 via 2 ones-matmuls
   over folded squared-videoT, -M*ml[l]*mv[v] via a K=1 matmul; +(b+M)
   applied as the ACT bias at evacuation. minsl = free-dim min; minsv =
   PE-transpose + free-dim min.

Schedule notes (engines are strict in-order queues; emission ~= order):
 - Three-stage pipeline per slot j: b2_evac(j-2) FIRST (its psum dep is
   the oldest, so ACT never head-blocks), then phase_a(j) [transposes +
   evacuations + squares], phase_b(j-1) [matmuls], b2_mins(j-2).
 - Video DMA: one SWDGE cast-load (f32->bf16) per batch; 1MB loads keep
   the Q7 descriptor generator ahead of the SDMA stream (2MB pair loads
   double Q7 DRAIN stalls; measured).
 - DMA order: pair-0 batches, lang(pair 0), mask rows, constants (the
   identity iota runs on gpsimd between descriptor gens), pair 1, rest
   of lang, pairs 2-7. Engine balance per pair: PE ~ transposes+matmuls,
   DVE ~ t0-evac+sq+folds+langT+mins, ACT ~ t1-evac+sq+lang-sq+masked.

Constraint honored: every DMA carries at most ONE semaphore wait, so
DMAs only write fresh tiles; marshalling is done by compute engines.
"""

import numpy as np

import concourse.bass as bass
import concourse.mybir as mybir
import concourse.tile as tile
from concourse import bacc, masks
from concourse.bass_utils import run_bass_kernel_spmd

N_CORES = 8
B, TV, TL, D = 128, 512, 64, 512
B_LOC = B // N_CORES  # 16
NP = B_LOC // 2  # 8
M_CONST = 32768.0

F32 = mybir.dt.float32
BF16 = mybir.dt.bfloat16
AX = mybir.AxisListType


def _emit(nc, tc, ctx, video, lang, mask_v, mask_l, out):
    TT = mybir.AluOpType
    AF = mybir.ActivationFunctionType

    consts = ctx.enter_context(tc.tile_pool(name="consts", bufs=1))
    vpool = ctx.enter_context(tc.tile_pool(name="vpool", bufs=1))
    vT = ctx.enter_context(tc.tile_pool(name="vT", bufs=3))
    langp = ctx.enter_context(tc.tile_pool(name="langp", bufs=4))
    sqs = ctx.enter_context(tc.tile_pool(name="sqs", bufs=4))
    smalls = ctx.enter_context(tc.tile_pool(name="smalls", bufs=4))
    maskedp = ctx.enter_context(tc.tile_pool(name="maskedp", bufs=4))
    ps_vT = ctx.enter_context(tc.tile_pool(name="ps_vT", bufs=3, space="PSUM"))
    ps_main = ctx.enter_context(tc.tile_pool(name="ps_main", bufs=3, space="PSUM"))
    ps_small = ctx.enter_context(tc.tile_pool(name="ps_small", bufs=2, space="PSUM"))

    # ---- DMA: first pair's batches first, identity right behind ----
    vid = []

    vp0 = vpool.tile([128, 8, 512], BF16, tag="vidp0")
    nc.gpsimd.dma_start(
        out=vp0[:, 0:4], in_=video[0].rearrange("(s p) d -> p s d", p=128)
    )
    nc.gpsimd.dma_start(
        out=vp0[:, 4:8], in_=video[1].rearrange("(s p) d -> p s d", p=128)
    )
    vid.append(vp0)

    def _load_pair(j):
        t = vpool.tile([128, 8, 512], BF16, tag=f"vidp{j}")
        for tt in range(2):
            nc.gpsimd.dma_start(
                out=t[:, 4 * tt : 4 * tt + 4],
                in_=video[2 * j + tt].rearrange("(s p) d -> p s d", p=128),
            )
        vid.append(t)

    lang_bf = consts.tile([128, NP, 512], BF16)
    lang_pairs_src = lang.rearrange("(j two) l d -> (two l) j d", two=2)
    nc.gpsimd.dma_start(out=lang_bf[:, 0:1, :], in_=lang_pairs_src[:, 0:1, :])
    maskv_rows = consts.tile([1, B_LOC, 512], BF16)
    nc.gpsimd.dma_start(
        out=maskv_rows[:], in_=mask_v.rearrange("(o b) v -> o b v", o=1)
    )
    maskl_rows = consts.tile([1, B_LOC, 64], BF16)
    nc.gpsimd.dma_start(
        out=maskl_rows[:], in_=mask_l.rearrange("(o b) l -> o b l", o=1)
    )

    # constants (identity iota on gpsimd runs after the gens above)
    identf = consts.tile([128, 128], F32)
    masks.make_identity(nc, identf[:])
    identb = consts.tile([128, 128], BF16)
    masks.make_identity(nc, identb[:])
    ones128 = consts.tile([128, 1], F32)
    nc.vector.memset(ones128[:], 1.0)
    m_col = consts.tile([128, 1], F32)
    nc.vector.memset(m_col[:], M_CONST)
    ones_mat = consts.tile([128, 64], BF16)
    nc.vector.memset(ones_mat[:], 1.0)
    ones_top = consts.tile([128, 1], F32)
    nc.vector.memset(ones_top[:], 0.0)
    nc.vector.memset(ones_top[0:64], 1.0)
    ones_bot = consts.tile([128, 1], F32)
    nc.vector.memset(ones_bot[:], 0.0)
    nc.vector.memset(ones_bot[64:128], 1.0)

    _load_pair(1)
    nc.gpsimd.dma_start(out=lang_bf[:, 1:NP, :], in_=lang_pairs_src[:, 1:NP, :])

    for j in range(2, NP):
        _load_pair(j)

    # masks natural layouts (HWDGE f32)
    maskv_nat = consts.tile([B_LOC, 512], F32)
    nc.sync.dma_start(out=maskv_nat[:], in_=mask_v)
    maskl_pair_nat = consts.tile([NP, 128], F32)
    nc.sync.dma_start(
        out=maskl_pair_nat[:], in_=mask_l.rearrange("(j two) l -> j (two l)", two=2)
    )
    mvc_ps = ps_small.tile([128, 4, B_LOC], F32, tag="ps_sm")
    for s in range(4):
        nc.tensor.transpose(
            mvc_ps[:, s],
            maskv_nat[:, 128 * s : 128 * (s + 1)],
            identf[0:B_LOC, 0:B_LOC],
        )
    maskv_cols = consts.tile([128, 4, B_LOC], F32)
    nc.vector.tensor_copy(maskv_cols[:], mvc_ps[:])
    mlc_ps = ps_small.tile([128, NP], F32, tag="ps_sm")
    nc.tensor.transpose(mlc_ps[:], maskl_pair_nat[:], identf[0:NP, 0:NP])
    masklT_pair = consts.tile([128, NP], F32)
    nc.vector.tensor_copy(masklT_pair[:], mlc_ps[:])

    # mask-only normalizers
    nv_sums = consts.tile([128, B_LOC], F32)
    nc.vector.tensor_reduce(
        nv_sums[:],
        maskv_cols[:].rearrange("p s b -> p b s"),
        axis=AX.X,
        op=TT.add,
    )
    red_nv = ps_small.tile([1, B_LOC], F32, tag="ps_sm")
    nc.tensor.matmul(red_nv[:], ones128[:], nv_sums[:], start=True, stop=True)
    rv = smalls.tile([1, B_LOC], F32, tag="rv")
    nc.vector.reciprocal(rv[:], red_nv[:])
    red_nl_e = ps_small.tile([1, NP], F32, tag="ps_sm")
    nc.tensor.matmul(red_nl_e[:], ones_top[:], masklT_pair[:], start=True, stop=True)
    rl_e = smalls.tile([1, NP], F32, tag="rl_e")
    nc.vector.reciprocal(rl_e[:], red_nl_e[:])
    red_nl_o = ps_small.tile([1, NP], F32, tag="ps_sm")
    nc.tensor.matmul(red_nl_o[:], ones_bot[:], masklT_pair[:], start=True, stop=True)
    rl_o = smalls.tile([1, NP], F32, tag="rl_o")
    nc.vector.reciprocal(rl_o[:], red_nl_o[:])

    negm_rows = consts.tile([1, B_LOC, 64], BF16)
    nc.vector.tensor_scalar_mul(negm_rows[:], maskl_rows[:], -M_CONST)

    minsv_all = consts.tile([128, B_LOC, 4], BF16)
    minsl_pairs = consts.tile([128, NP], F32)
    b_pairs = consts.tile([128, NP], F32)
    bias_pairs = consts.tile([128, NP], F32)

    def phase_a(j):
        # video transposes + evacuations (t0 -> DVE, t1 -> ACT)
        vt_sb = vT.tile([128, 8, 512], BF16, tag="vt_sb")
        for t in range(2):
            for g in range(2):
                vt_ps = ps_vT.tile([128, 2, 512], BF16, tag="vt_ps")
                for kk in range(2):
                    k = 2 * g + kk
                    for s in range(4):
                        nc.tensor.transpose(
                            vt_ps[:, kk, 128 * s : 128 * (s + 1)],
                            vid[j][:, 4 * t + s, 128 * k : 128 * (k + 1)],
                            identb[:],
                        )
                if t == 0:
                    nc.vector.tensor_copy(
                        vt_sb[:, 2 * g : 2 * g + 2], vt_ps[:]
                    )
                else:
                    nc.scalar.copy(vt_sb[:, 4 + 2 * g : 4 + 2 * g + 2], vt_ps[:])

        # lang: squares+accum on ACT, bias col, langT transposes + DVE -2x
        sq_l = sqs.tile([128, 512], BF16, tag="sq_l")
        nc.scalar.activation(
            sq_l[:], lang_bf[:, j], AF.Square, accum_out=b_pairs[:, j : j + 1]
        )
        nc.scalar.activation(
            bias_pairs[:, j : j + 1],
            b_pairs[:, j : j + 1],
            AF.Identity,
            bias=m_col[:],
        )
        lg_ps = ps_small.tile([128, 4, 128], BF16, tag="ps_sm")
        for k in range(4):
            nc.tensor.transpose(
                lg_ps[:, k], lang_bf[:, j, 128 * k : 128 * (k + 1)], identb[:]
            )
        langT = langp.tile([128, 4, 128], BF16, tag="langT")
        nc.vector.tensor_scalar_mul(langT[:], lg_ps[:], -2.0)

        # squares: t0 on DVE (one FD2048 op), t1 on ACT; folds on DVE
        sq0 = sqs.tile([128, 4, 512], BF16, tag="sq0")
        nc.vector.tensor_tensor(sq0[:], vt_sb[:, 0:4], vt_sb[:, 0:4], op=TT.mult)
        sq1 = sqs.tile([128, 4, 512], BF16, tag="sq1")
        nc.scalar.activation(sq1[:], vt_sb[:, 4:8], AF.Square)
        sqh = sqs.tile([128, 2, 2, 512], BF16, tag="sqh")
        nc.vector.tensor_tensor(sqh[:, 0], sq0[:, 0:2], sq0[:, 2:4], op=TT.add)
        nc.vector.tensor_tensor(sqh[:, 1], sq1[:, 0:2], sq1[:, 2:4], op=TT.add)
        return langT, vt_sb, sqh

    def phase_b(j, actx):
        langT, vt_sb, sqh = actx
        psum_pair = ps_main.tile([128, 512], F32, tag="psum_T")
        halves = [psum_pair[0:64, :], psum_pair[64:128, :]]
        for k in range(4):
            for t in range(2):
                nc.tensor.matmul(
                    halves[t],
                    langT[:, k, 64 * t : 64 * (t + 1)],
                    vt_sb[:, 4 * t + k],
                    start=(k == 0),
                    stop=False,
                    skip_group_check=True,
                )
        for k in range(2):
            for t in range(2):
                nc.tensor.matmul(
                    halves[t],
                    ones_mat[:],
                    sqh[:, t, k],
                    start=False,
                    stop=False,
                    skip_group_check=True,
                )
        for t in range(2):
            nc.tensor.matmul(
                halves[t],
                negm_rows[:, 2 * j + t],
                maskv_rows[:, 2 * j + t],
                start=False,
                stop=True,
                skip_group_check=True,
            )
        return psum_pair

    def b2_evac(j, psum_pair):
        masked_pr = maskedp.tile([128, 512], BF16, tag="masked_pr")
        nc.scalar.activation(
            masked_pr[:],
            psum_pair[:],
            AF.Identity,
            bias=bias_pairs[:, j : j + 1],
            scale=1.0,
        )
        return masked_pr

    def b2_mins(j, masked_pr):
        nc.vector.tensor_reduce(
            minsl_pairs[:, j : j + 1], masked_pr[:], axis=AX.X, op=TT.min
        )
        o2 = ps_small.tile([128, 4, 2, 64], BF16, tag="ps_sm")
        for s in range(4):
            nc.tensor.transpose(
                o2[:, s], masked_pr[:, 128 * s : 128 * (s + 1)], identb[:]
            )
        nc.vector.tensor_reduce(
            minsv_all[:, 2 * j : 2 * j + 2, :].rearrange("p t s -> p s t"),
            o2[:],
            axis=AX.X,
            op=TT.min,
        )

    # slot order: b2_evac(j-2), a(j), b(j-1), b2_mins(j-2)
    mv_mask = consts.tile([128, B_LOC, 4], F32)
    mv_sums = consts.tile([128, B_LOC], F32)
    mlm = consts.tile([128, NP], F32)
    mvcT = maskv_cols[:].rearrange("p s b -> p b s")

    def final_partial(b0, b1):
        # masked minsv/minsl partial reductions for batches [b0:b1)
        nc.vector.tensor_tensor(
            mv_mask[:, b0:b1], minsv_all[:, b0:b1], mvcT[:, b0:b1], op=TT.mult
        )
        nc.vector.tensor_reduce(
            mv_sums[:, b0:b1], mv_mask[:, b0:b1], axis=AX.X, op=TT.add
        )
        nc.vector.tensor_tensor(
            mlm[:, b0 // 2 : b1 // 2],
            minsl_pairs[:, b0 // 2 : b1 // 2],
            masklT_pair[:, b0 // 2 : b1 // 2],
            op=TT.mult,
        )

    actx, bctx, mctx = {}, {}, {}
    for j in range(NP + 2):
        if j >= 2:
            mctx[j - 2] = b2_evac(j - 2, bctx.pop(j - 2))
        if j < NP:
            actx[j] = phase_a(j)
        if j >= 2:
            # minsv transposes ahead of the matmul block: their masked-pair
            # dep is ready at slot start, releasing the DVE mins earlier
            b2_mins(j - 2, mctx.pop(j - 2))
        if 1 <= j <= NP:
            bctx[j - 1] = phase_b(j - 1, actx.pop(j - 1))

    # ---- final: collectors + global reductions ----
    final_partial(0, B_LOC)

    red_mv = ps_main.tile([1, B_LOC], F32, tag="psum_T")
    nc.tensor.matmul(red_mv[:], ones128[:], mv_sums[:], start=True, stop=True)
    t1 = smalls.tile([1, B_LOC], F32, tag="t1")
    nc.vector.tensor_tensor(t1[:], red_mv[:], rv[:], op=TT.mult)

    t2 = smalls.tile([1, B_LOC], F32, tag="t2")
    t2v = t2[:].rearrange("a (jj two) -> a jj two", two=2)

    red_ml_e = ps_main.tile([1, NP], F32, tag="psum_T")
    nc.tensor.matmul(red_ml_e[:], ones_top[:], mlm[:], start=True, stop=True)
    nc.vector.tensor_tensor(t2v[:, :, 0], red_ml_e[:], rl_e[:], op=TT.mult)

    red_ml_o = ps_main.tile([1, NP], F32, tag="psum_T")
    nc.tensor.matmul(red_ml_o[:], ones_bot[:], mlm[:], start=True, stop=True)
    nc.vector.tensor_tensor(t2v[:, :, 1], red_ml_o[:], rl_o[:], op=TT.mult)

    out_sb = smalls.tile([1, B_LOC], F32, tag="out_sb")
    nc.vector.tensor_tensor(out_sb[:], t1[:], t2[:], op=TT.add)
    nc.sync.dma_start(out=out[:], in_=out_sb[:])


_CACHED_NC = None


def _get_nc():
    global _CACHED_NC
    if _CACHED_NC is None:
        from contextlib import ExitStack

        nc = bacc.Bacc(
            "TRN2", target_bir_lowering=False, debug=False, num_devices=N_CORES
        )
        video = nc.dram_tensor(
            "video", [B_LOC, TV, D], F32, kind="ExternalInput"
        ).ap()
        lang = nc.dram_tensor("lang", [B_LOC, TL, D], F32, kind="ExternalInput").ap()
        mask_v = nc.dram_tensor(
            "mask_v", [B_LOC, TV], F32, kind="ExternalInput"
        ).ap()
        mask_l = nc.dram_tensor(
            "mask_l", [B_LOC, TL], F32, kind="ExternalInput"
        ).ap()
        out = nc.dram_tensor("out", [1, B_LOC], F32, kind="ExternalOutput").ap()
        with tile.TileContext(nc) as tc:
            with ExitStack() as ctx:
                _emit(nc, tc, ctx, video, lang, mask_v, mask_l, out)
        nc.compile()
        _CACHED_NC = nc
    return _CACHED_NC


def _run(video_feat, lang_feat, mask_v, mask_l, trace=False):
    nc = _get_nc()
    video_feat = np.ascontiguousarray(video_feat, dtype=np.float32)
    lang_feat = np.ascontiguousarray(lang_feat, dtype=np.float32)
    mask_v = np.ascontiguousarray(mask_v, dtype=np.float32)
    mask_l = np.ascontiguousarray(mask_l, dtype=np.float32)
    in_maps = []
    for c in range(N_CORES):
        sl = slice(c * B_LOC, (c + 1) * B_LOC)
        in_maps.append(
            {
                "video": video_feat[sl],
                "lang": lang_feat[sl],
                "mask_v": mask_v[sl],
                "mask_l": mask_l[sl],
            }
        )
    res = run_bass_kernel_spmd(nc, in_maps, list(range(N_CORES)), trace=trace)
    full = np.concatenate(
        [res.results[c]["out"].reshape(-1) for c in range(N_CORES)]
    ).astype(np.float32)
    return full, res


def kernel(video_feat, lang_feat, mask_v, mask_l):
    out, _ = _run(video_feat, lang_feat, mask_v, mask_l, trace=False)
    return out
